# revision 4
# baseline (speedup 1.0000x reference)
"""CBDice loss kernel for 8 TRN2 NeuronCores (Bass/Tile, bf16 pipeline).

Strategy
--------
Data parallel: 32 images sharded 4-per-core across 8 cores.

Per image and per tensor X in {pred, target} (all on-chip values bf16):
    e_0 = X;  e_t = erode3x3(e_{t-1})  t = 1..16      (shared erosion chain)
    d_t = dilate3x3(e_t)               t = 1..11
    soft_skeleton(X) = 1 - prod_{t=0..10} (1 - (e_t - d_{t+1}))
        (the reference's relu and clips are mathematically no-ops:
         opening is anti-extensive, and the recurrence stays in [0,1])
    radius(X) = (sum_{t=1..16} e_t / 16) * X          (clip is a no-op)
and per-image sums (f32, via ACT accum_out):
    s1 = sum(skelP*G*rG), s2 = sum(skelP*rG),
    s3 = sum(skelG*P*rP), s4 = sum(skelG*rP),
    c-sums of skel and skel*mask for exact EPS correction.
Final scalar math on host (f64).

Layout: one image-tensor = SBUF tile [128 partitions, 8 slots, 770 cols]
    slot s <-> image row 6p + s - 1 (slots 1..6 = data, 0/7 = halo rows)
    col  c <-> image col c - 1      (cols 1..768 = data, 0/769 = pads)
Halo rows / pad cols hold replicate padding (equivalent to the
reference's +-inf pooling pads for one 3x3 window). A 3x3 pool is 4 DVE
tensor_tensor ops (separable 3-taps, pairwise, bf16 2x mode). Halo rows
are refreshed after each erosion with 4 SBUF->SBUF DMAs (compute engines
cannot cross partitions); pad cols with 2 tiny ACT copies. ACT handles
all 1-input affine/copy/accum work. Host pre-builds the padded bf16
tiles so the device does zero layout work.
"""
import numpy as np
import ml_dtypes

import concourse.bass as bass
from concourse import bacc
import concourse.tile as tile
import concourse.mybir as mybir
from concourse.bass_utils import run_bass_kernel_spmd

A = mybir.AluOpType
F = mybir.ActivationFunctionType
dt = mybir.dt
BF16 = ml_dtypes.bfloat16

EPS = 1e-6
SKEL_ITERS = 10
RADIUS_ITERS = 16
H = W = 768
P_DIM = 128
FOLD = 6
NSLOT = 8
WS = 770
TILE_F = NSLOT * WS
NSUM = 10  # per-image sum columns (8 used)


def _ap(t, s0, nslots, c0=0, w=WS):
    """[(nslots),(w)] view of a [128, k*WS]-flat tile starting at slot s0, col c0."""
    if c0 == 0 and w == WS:
        return t[:, s0 * WS:(s0 + nslots) * WS]
    v = t[:, s0 * WS + c0: s0 * WS + c0 + 1]
    return bass.AP(tensor=v.tensor, offset=v.offset,
                   ap=[list(t[:, :].ap[0]), [WS, nslots], [1, w]])


def build(n_img=4, skel_iters=SKEL_ITERS, radius_iters=RADIUS_ITERS, repeat=1):
    nc = bacc.Bacc("TRN2", target_bir_lowering=False, debug=False, num_devices=8)
    pred_d = nc.dram_tensor("pred", [n_img, P_DIM, NSLOT, WS], dt.bfloat16,
                            kind="ExternalInput").ap()
    targ_d = nc.dram_tensor("targ", [n_img, P_DIM, NSLOT, WS], dt.bfloat16,
                            kind="ExternalInput").ap()
    sums_d = nc.dram_tensor("sums", [P_DIM, n_img * NSUM], dt.float32,
                            kind="ExternalOutput").ap()
    n_dil = skel_iters + 1

    with tile.TileContext(nc) as tc:
        with tc.tile_pool(name="pers", bufs=1) as pers, \
             tc.tile_pool(name="dbl", bufs=2) as dbl, \
             tc.tile_pool(name="fin", bufs=3) as finp:
            sums = pers.tile([P_DIM, n_img * NSUM], dt.float32)
            nc.vector.memset(sums[:], 0.0)

            import contextlib
            rep_ctx = tc.For_i(0, repeat, 1) if repeat > 1 else contextlib.nullcontext()

            def pool3(e_src, out_data_ap, op):
                """3x3 pool of 8-slot tile e_src -> 6-slot data region out.

                scrA (7 slots) holds A then B; scrT (6 slots) the H result.
                """
                scrA = dbl.tile([P_DIM, 7 * WS], dt.bfloat16, tag="scrA")
                scrT = dbl.tile([P_DIM, 6 * WS], dt.bfloat16, tag="scrT")
                # H pass (full slot width so pad cols propagate)
                nc.vector.tensor_tensor(_ap(scrA, 0, 7), _ap(e_src, 0, 7),
                                        _ap(e_src, 1, 7), op)
                nc.vector.tensor_tensor(_ap(scrT, 0, 6), _ap(scrA, 0, 6),
                                        _ap(e_src, 2, 6), op)
                # W pass: B into scrA slots 0..5, then combine
                nc.vector.tensor_tensor(_ap(scrA, 0, 6, 0, 769),
                                        _ap(scrT, 0, 6, 0, 769),
                                        _ap(scrT, 0, 6, 1, 769), op)
                nc.vector.tensor_tensor(out_data_ap,
                                        _ap(scrA, 0, 6, 0, 768),
                                        _ap(scrA, 0, 6, 1, 768), op)

            def refresh(e):
                """Restore pad cols (ACT) and halo rows (DMA) of chain tile e."""
                nc.scalar.copy(_ap(e, 1, 6, 0, 1), _ap(e, 1, 6, 1, 1))
                nc.scalar.copy(_ap(e, 1, 6, 769, 1), _ap(e, 1, 6, 768, 1))
                nc.gpsimd.dma_start(_ap(e, 0, 1)[1:128, :], _ap(e, 6, 1)[0:127, :])
                nc.gpsimd.dma_start(_ap(e, 7, 1)[0:127, :], _ap(e, 1, 1)[1:128, :])
                nc.gpsimd.dma_start(_ap(e, 0, 1)[0:1, :], _ap(e, 1, 1)[0:1, :])
                nc.gpsimd.dma_start(_ap(e, 7, 1)[127:128, :], _ap(e, 6, 1)[127:128, :])

            with rep_ctx:
              for n in range(n_img):
                xP = pers.tile([P_DIM, TILE_F], dt.bfloat16, tag="xP")
                nc.sync.dma_start(xP[:], pred_d[n].rearrange("p s w -> p (s w)"))
                xG = pers.tile([P_DIM, TILE_F], dt.bfloat16, tag="xG")
                nc.sync.dma_start(xG[:], targ_d[n].rearrange("p s w -> p (s w)"))

                side = {}
                for name, x0 in (("P", xP), ("G", xG)):
                    q = pers.tile([P_DIM, 6 * WS], dt.bfloat16, tag=f"q{name}")
                    acc = pers.tile([P_DIM, 6 * WS], dt.bfloat16, tag=f"acc{name}")
                    e_prev = x0
                    for t in range(1, radius_iters + 1):
                        e_new = dbl.tile([P_DIM, TILE_F], dt.bfloat16, tag="ech")
                        pool3(e_prev, _ap(e_new, 1, 6, 1, 768), A.min)
                        refresh(e_new)

                        if t <= n_dil:
                            d = pers.tile([P_DIM, 6 * WS], dt.bfloat16, tag="d")
                            pool3(e_new, _ap(d, 0, 6, 1, 768), A.max)
                            t1 = dbl.tile([P_DIM, 6 * WS], dt.bfloat16, tag="pw")
                            nc.vector.tensor_tensor(_ap(t1, 0, 6, 1, 768),
                                                    _ap(d, 0, 6, 1, 768),
                                                    _ap(e_prev, 1, 6, 1, 768),
                                                    A.subtract)
                            if t == 1:
                                nc.scalar.activation(_ap(q, 0, 6, 1, 768),
                                                     _ap(t1, 0, 6, 1, 768),
                                                     F.Copy, bias=1.0, scale=1.0)
                            else:
                                u = dbl.tile([P_DIM, 6 * WS], dt.bfloat16, tag="pw")
                                nc.scalar.activation(_ap(u, 0, 6, 1, 768),
                                                     _ap(t1, 0, 6, 1, 768),
                                                     F.Copy, bias=1.0, scale=1.0)
                                nc.vector.tensor_tensor(_ap(q, 0, 6, 1, 768),
                                                        _ap(q, 0, 6, 1, 768),
                                                        _ap(u, 0, 6, 1, 768), A.mult)
                        if t == 1:
                            nc.scalar.copy(_ap(acc, 0, 6, 1, 768),
                                           _ap(e_new, 1, 6, 1, 768))
                        else:
                            nc.vector.tensor_tensor(_ap(acc, 0, 6, 1, 768),
                                                    _ap(acc, 0, 6, 1, 768),
                                                    _ap(e_new, 1, 6, 1, 768), A.add)
                        e_prev = e_new

                    side[name] = (x0, q, acc)

                # ---- finals for this image ----
                base = n * NSUM
                scr = finp.tile([P_DIM, 6 * WS], dt.bfloat16, tag="fin")
                res = {}
                for name in ("P", "G"):
                    x0, q, acc = side[name]
                    # skel = 1 - q (into a dbl "pw" slot), c0 = sum(skel)
                    skel = dbl.tile([P_DIM, 6 * WS], dt.bfloat16, tag="pw")
                    c0_col = base + (4 if name == "P" else 6)
                    nc.scalar.activation(_ap(skel, 0, 6, 1, 768), _ap(q, 0, 6, 1, 768),
                                         F.Copy, bias=1.0, scale=-1.0,
                                         accum_out=sums[:, c0_col:c0_col + 1])
                    # r = (acc/16) * x, written over acc's slot
                    rsc = finp.tile([P_DIM, 6 * WS], dt.bfloat16, tag="fin")
                    nc.scalar.activation(_ap(rsc, 0, 6, 1, 768), _ap(acc, 0, 6, 1, 768),
                                         F.Copy, bias=0.0, scale=1.0 / radius_iters)
                    r = pers.tile([P_DIM, 6 * WS], dt.bfloat16, tag=f"acc{name}")
                    nc.vector.tensor_tensor(_ap(r, 0, 6, 1, 768), _ap(rsc, 0, 6, 1, 768),
                                            _ap(x0, 1, 6, 1, 768), A.mult)
                    res[name] = (x0, skel, r)

                xP0, skelP, rP = res["P"]
                xG0, skelG, rG = res["G"]
                for k, (skel, other_x, r, c1_col) in enumerate(
                        [(skelP, xG0, rG, base + 5), (skelG, xP0, rP, base + 7)]):
                    s_base = base + 2 * k
                    u1 = finp.tile([P_DIM, 6 * WS], dt.bfloat16, tag="fin")
                    nc.vector.tensor_tensor(_ap(u1, 0, 6, 1, 768), _ap(skel, 0, 6, 1, 768),
                                            _ap(other_x, 1, 6, 1, 768), A.mult)
                    z1 = finp.tile([P_DIM, 6 * WS], dt.bfloat16, tag="fin")
                    nc.vector.tensor_tensor(_ap(z1, 0, 6, 1, 768), _ap(u1, 0, 6, 1, 768),
                                            _ap(r, 0, 6, 1, 768), A.mult)
                    nc.scalar.activation(_ap(scr, 0, 6, 1, 768), _ap(z1, 0, 6, 1, 768),
                                         F.Copy, accum_out=sums[:, s_base:s_base + 1])
                    z2 = finp.tile([P_DIM, 6 * WS], dt.bfloat16, tag="fin")
                    nc.vector.tensor_tensor(_ap(z2, 0, 6, 1, 768), _ap(skel, 0, 6, 1, 768),
                                            _ap(r, 0, 6, 1, 768), A.mult)
                    nc.scalar.activation(_ap(scr, 0, 6, 1, 768), _ap(z2, 0, 6, 1, 768),
                                         F.Copy, accum_out=sums[:, s_base + 1:s_base + 2])
                    nc.scalar.activation(_ap(scr, 0, 6, 1, 768), _ap(u1, 0, 6, 1, 768),
                                         F.Copy, accum_out=sums[:, c1_col:c1_col + 1])

            out_t = pers.tile([P_DIM, n_img * NSUM], dt.float32)
            nc.vector.tensor_copy(out_t[:], sums[:])
            nc.sync.dma_start(sums_d[:], out_t[:])

    nc.compile()
    return nc


def host_layout(x):
    """(N,1,768,768) f32 -> (N,128,8,770) bf16 padded tiles."""
    n = x.shape[0]
    xb = np.ascontiguousarray(x.reshape(n, H, W)).astype(BF16)
    t = np.empty((n, P_DIM, NSLOT, WS), dtype=BF16)
    t[:, :, 1:7, 1:769] = xb.reshape(n, P_DIM, FOLD, W)
    t[:, 1:, 0, 1:769] = xb[:, 5::6][:, :127]      # halo row 6p-1
    t[:, 0, 0, 1:769] = xb[:, 0]                   # replicate row 0
    t[:, :127, 7, 1:769] = xb[:, 6::6]             # halo row 6p+6
    t[:, 127, 7, 1:769] = xb[:, 767]               # replicate row 767
    t[:, :, :, 0] = t[:, :, :, 1]                  # pad cols
    t[:, :, :, 769] = t[:, :, :, 768]
    return t


_NC_CACHE = {}


def _get_nc(n_img=4):
    if n_img not in _NC_CACHE:
        _NC_CACHE[n_img] = build(n_img=n_img)
    return _NC_CACHE[n_img]


def finish_host(all_sums):
    """all_sums: list of (128, n_img*NSUM) f32 per core -> scalar loss."""
    per_img = []
    for s in all_sums:
        v = s.astype(np.float64).sum(axis=0)
        for i in range(v.shape[0] // NSUM):
            per_img.append(v[i * NSUM:(i + 1) * NSUM])
    per_img = np.stack(per_img)
    s1, s2, s3, s4 = per_img[:, 0], per_img[:, 1], per_img[:, 2], per_img[:, 3]
    c0P, c1P, c0G, c1G = per_img[:, 4], per_img[:, 5], per_img[:, 6], per_img[:, 7]
    s1 = s1 + EPS * c1P
    s2 = s2 + EPS * c0P
    s3 = s3 + EPS * c1G
    s4 = s4 + EPS * c0G
    prec = s1 / (s2 + EPS)
    rec = s3 / (s4 + EPS)
    cb = 2.0 * prec * rec / (prec + rec + EPS)
    return np.float32(np.mean(1.0 - cb))


def kernel(pred, target):
    pred = np.asarray(pred)
    target = np.asarray(target)
    n_total = pred.shape[0]
    n_cores = 8
    n_img = n_total // n_cores
    nc = _get_nc(n_img)
    tp = host_layout(pred)
    tg = host_layout(target)
    in_maps = [{"pred": tp[c * n_img:(c + 1) * n_img],
                "targ": tg[c * n_img:(c + 1) * n_img]} for c in range(n_cores)]
    res = run_bass_kernel_spmd(nc, in_maps, list(range(n_cores)))
    return finish_host([r["sums"] for r in res.results])


# revision 11
# speedup vs baseline: 20.0502x; 20.0502x over previous
"""CBDice loss kernel for 8 TRN2 NeuronCores (Bass/Tile, bf16 pipeline).

Strategy
--------
Data parallel: 32 images sharded 4-per-core across 8 cores.

Per image and per tensor X in {pred, target} (all on-chip values bf16):
    e_0 = X;  e_t = erode3x3(e_{t-1})  t = 1..16      (shared erosion chain)
    d_t = dilate3x3(e_t)               t = 1..11
    soft_skeleton(X) = 1 - prod_{t=0..10} (1 - (e_t - d_{t+1}))
        (the reference's relu and clips are mathematically no-ops:
         opening is anti-extensive, and the recurrence stays in [0,1])
    radius(X) = (sum_{t=1..16} e_t / 16) * X          (clip is a no-op)
and per-image sums (f32, via ACT accum_out):
    s1 = sum(skelP*G*rG), s2 = sum(skelP*rG),
    s3 = sum(skelG*P*rP), s4 = sum(skelG*rP),
    c-sums of skel and skel*mask for exact EPS correction.
Final scalar math on host (f64).

Layout: one image-tensor = SBUF tile [128 partitions, 8 slots, 770 cols]
    slot s <-> image row 6p + s - 1 (slots 1..6 = data, 0/7 = halo rows)
    col  c <-> image col c - 1      (cols 1..768 = data, 0/769 = pads)
Halo rows / pad cols hold replicate padding (equivalent to the
reference's +-inf pooling pads for one 3x3 window). A 3x3 pool is 4 DVE
tensor_tensor ops (separable 3-taps, pairwise, bf16 2x mode). Halo rows
are refreshed after each erosion with 4 SBUF->SBUF DMAs (compute engines
cannot cross partitions); pad cols with 2 tiny ACT copies. ACT handles
all 1-input affine/copy/accum work. Host pre-builds the padded bf16
tiles so the device does zero layout work.
"""
import numpy as np
import ml_dtypes

import concourse.bass as bass
from concourse import bacc
import concourse.tile as tile
import concourse.mybir as mybir
from concourse.bass_utils import run_bass_kernel_spmd

A = mybir.AluOpType
F = mybir.ActivationFunctionType
dt = mybir.dt
BF16 = ml_dtypes.bfloat16

EPS = 1e-6
SKEL_ITERS = 10
RADIUS_ITERS = 16
H = W = 768
P_DIM = 128
FOLD = 6
NSLOT = 8
WS = 770
TILE_F = NSLOT * WS
NSUM = 10  # per-image sum columns (8 used)


def _ap(t, s0, nslots, c0=0, w=WS):
    """[(nslots),(w)] view of a [128, k*WS]-flat tile starting at slot s0, col c0."""
    if c0 == 0 and w == WS:
        return t[:, s0 * WS:(s0 + nslots) * WS]
    v = t[:, s0 * WS + c0: s0 * WS + c0 + 1]
    return bass.AP(tensor=v.tensor, offset=v.offset,
                   ap=[list(t[:, :].ap[0]), [WS, nslots], [1, w]])


def _ap_pair(t, sa, sb, c0, w):
    """[(2 slots sa,sb),(w)] strided view (sb>sa)."""
    v = t[:, sa * WS + c0: sa * WS + c0 + 1]
    return bass.AP(tensor=v.tensor, offset=v.offset,
                   ap=[list(t[:, :].ap[0]), [(sb - sa) * WS, 2], [1, w]])


def build(n_img=4, skel_iters=SKEL_ITERS, radius_iters=RADIUS_ITERS, repeat=1,
          skip_refresh=False, skip_skel=False, skip_acc=False, halo_hwdge=True):
    nc = bacc.Bacc("TRN2", target_bir_lowering=False, debug=False, num_devices=8)
    pred_d = nc.dram_tensor("pred", [n_img, P_DIM, NSLOT, WS], dt.bfloat16,
                            kind="ExternalInput").ap()
    targ_d = nc.dram_tensor("targ", [n_img, P_DIM, NSLOT, WS], dt.bfloat16,
                            kind="ExternalInput").ap()
    sums_d = nc.dram_tensor("sums", [P_DIM, n_img * NSUM], dt.float32,
                            kind="ExternalOutput").ap()
    n_dil = skel_iters + 1

    with tile.TileContext(nc) as tc:
        with tc.tile_pool(name="pers", bufs=1) as pers, \
             tc.tile_pool(name="dbl", bufs=2) as dbl, \
             tc.tile_pool(name="ech4", bufs=4) as ech4:
            sums = pers.tile([P_DIM, n_img * NSUM], dt.float32)
            nc.vector.memset(sums[:], 0.0)

            import contextlib
            rep_ctx = tc.For_i(0, repeat, 1) if repeat > 1 else contextlib.nullcontext()

            def pool3(e_src, out_data_ap, op, out_boundary_ap=None):
                """3x3 pool of 8-slot tile -> 6-slot data region; scrA holds
                A (7 slots), then tmp (in-place), then B (in-place fwd-shift)."""
                scrA = dbl.tile([P_DIM, 7 * WS], dt.bfloat16, tag="scrA")
                nc.vector.tensor_tensor(_ap(scrA, 0, 7), _ap(e_src, 0, 7),
                                        _ap(e_src, 1, 7), op)
                nc.vector.tensor_tensor(_ap(scrA, 0, 6), _ap(scrA, 0, 6),
                                        _ap(e_src, 2, 6), op)
                nc.vector.tensor_tensor(_ap(scrA, 0, 6, 0, 769),
                                        _ap(scrA, 0, 6, 0, 769),
                                        _ap(scrA, 0, 6, 1, 769), op)
                if out_boundary_ap is not None:
                    # boundary out slots first so halo DMAs launch early
                    nc.vector.tensor_tensor(out_boundary_ap,
                                            _ap_pair(scrA, 0, 5, 0, 768),
                                            _ap_pair(scrA, 0, 5, 1, 768), op)
                    nc.vector.tensor_tensor(out_data_ap,
                                            _ap(scrA, 1, 4, 0, 768),
                                            _ap(scrA, 1, 4, 1, 768), op)
                else:
                    nc.vector.tensor_tensor(out_data_ap,
                                            _ap(scrA, 0, 6, 0, 768),
                                            _ap(scrA, 0, 6, 1, 768), op)

            dma_eng = nc.sync if halo_hwdge else nc.gpsimd

            def refresh(e):
                """Restore pad cols (ACT) and halo rows (DMA) of chain tile e.

                Boundary slots {1,6} get pads first; the global-edge halo rows
                come from partition-aligned ACT copies (later overwritten on
                interior partitions by the big DMAs); the two big halo DMAs go
                out on different HWDGE queues (SP + ACT)."""
                if skip_refresh:
                    return
                nc.scalar.copy(_ap_pair(e, 1, 6, 0, 1), _ap_pair(e, 1, 6, 1, 1))
                nc.scalar.copy(_ap_pair(e, 1, 6, 769, 1), _ap_pair(e, 1, 6, 768, 1))
                nc.scalar.copy(_ap(e, 0, 1)[0:32, :], _ap(e, 1, 1)[0:32, :])
                nc.scalar.copy(_ap(e, 7, 1)[96:128, :], _ap(e, 6, 1)[96:128, :])
                nc.sync.dma_start(_ap(e, 0, 1)[1:128, :], _ap(e, 6, 1)[0:127, :])
                nc.scalar.dma_start(_ap(e, 7, 1)[0:127, :], _ap(e, 1, 1)[1:128, :])
                # interior pad cols only feed the next H pass
                nc.scalar.copy(_ap(e, 2, 4, 0, 1), _ap(e, 2, 4, 1, 1))
                nc.scalar.copy(_ap(e, 2, 4, 769, 1), _ap(e, 2, 4, 768, 1))

            with rep_ctx:
              for n in range(n_img):
                xP = pers.tile([P_DIM, TILE_F], dt.bfloat16, tag="xP")
                nc.sync.dma_start(xP[:], pred_d[n].rearrange("p s w -> p (s w)"))
                xG = pers.tile([P_DIM, TILE_F], dt.bfloat16, tag="xG")
                nc.sync.dma_start(xG[:], targ_d[n].rearrange("p s w -> p (s w)"))

                st = {}
                for name, x0 in (("P", xP), ("G", xG)):
                    q_t = pers.tile([P_DIM, 6 * WS], dt.bfloat16, tag=f"q{name}")
                    acc_t = pers.tile([P_DIM, 6 * WS], dt.bfloat16, tag=f"acc{name}")
                    st[name] = {"x0": x0, "q": q_t, "acc": acc_t, "e_prev": x0}

                for t in range(1, radius_iters + 1):
                    for name in ("P", "G"):
                        S = st[name]
                        e_prev = S["e_prev"]
                        e_new = ech4.tile([P_DIM, TILE_F], dt.bfloat16, tag="ech")
                        pool3(e_prev, _ap(e_new, 2, 4, 1, 768), A.min,
                              out_boundary_ap=_ap_pair(e_new, 1, 6, 1, 768))
                        refresh(e_new)

                        if t <= n_dil and not skip_skel:
                            d = dbl.tile([P_DIM, 6 * WS], dt.bfloat16, tag="d")
                            pool3(e_new, _ap(d, 1, 4, 1, 768), A.max,
                                  out_boundary_ap=_ap_pair(d, 0, 5, 1, 768))
                            t1 = dbl.tile([P_DIM, 6 * WS], dt.bfloat16, tag="pw")
                            nc.vector.tensor_tensor(_ap(t1, 0, 6, 1, 768),
                                                    _ap(d, 0, 6, 1, 768),
                                                    _ap(e_prev, 1, 6, 1, 768),
                                                    A.subtract)
                            q = S["q"]
                            if t == 1:
                                nc.scalar.activation(_ap(q, 0, 6, 1, 768),
                                                     _ap(t1, 0, 6, 1, 768),
                                                     F.Copy, bias=1.0, scale=1.0)
                            else:
                                u = dbl.tile([P_DIM, 6 * WS], dt.bfloat16, tag="pw")
                                nc.scalar.activation(_ap(u, 0, 6, 1, 768),
                                                     _ap(t1, 0, 6, 1, 768),
                                                     F.Copy, bias=1.0, scale=1.0)
                                nc.vector.tensor_tensor(_ap(q, 0, 6, 1, 768),
                                                        _ap(q, 0, 6, 1, 768),
                                                        _ap(u, 0, 6, 1, 768), A.mult)
                        acc = S["acc"]
                        if skip_acc:
                            pass
                        elif t == 1:
                            nc.scalar.copy(_ap(acc, 0, 6, 1, 768),
                                           _ap(e_new, 1, 6, 1, 768))
                        else:
                            nc.vector.tensor_tensor(_ap(acc, 0, 6, 1, 768),
                                                    _ap(acc, 0, 6, 1, 768),
                                                    _ap(e_new, 1, 6, 1, 768), A.add)
                        S["e_prev"] = e_new

                # ---- finals for this image ----
                base = n * NSUM
                scr = dbl.tile([P_DIM, 6 * WS], dt.bfloat16, tag="d")
                res = {}
                for name in ("P", "G"):
                    S = st[name]
                    x0, q, acc = S["x0"], S["q"], S["acc"]
                    skel = dbl.tile([P_DIM, 6 * WS], dt.bfloat16, tag="pw")
                    c0_col = base + (4 if name == "P" else 6)
                    nc.scalar.activation(_ap(skel, 0, 6, 1, 768), _ap(q, 0, 6, 1, 768),
                                         F.Copy, bias=1.0, scale=-1.0,
                                         accum_out=sums[:, c0_col:c0_col + 1])
                    rsc = dbl.tile([P_DIM, 6 * WS], dt.bfloat16, tag="fin")
                    nc.scalar.activation(_ap(rsc, 0, 6, 1, 768), _ap(acc, 0, 6, 1, 768),
                                         F.Copy, bias=0.0, scale=1.0 / radius_iters)
                    r = pers.tile([P_DIM, 6 * WS], dt.bfloat16, tag=f"acc{name}")
                    nc.vector.tensor_tensor(_ap(r, 0, 6, 1, 768), _ap(rsc, 0, 6, 1, 768),
                                            _ap(x0, 1, 6, 1, 768), A.mult)
                    res[name] = (x0, skel, r)

                xP0, skelP, rP = res["P"]
                xG0, skelG, rG = res["G"]
                for k, (skel, other_x, r, c1_col) in enumerate(
                        [(skelP, xG0, rG, base + 5), (skelG, xP0, rP, base + 7)]):
                    s_base = base + 2 * k
                    u1 = dbl.tile([P_DIM, 6 * WS], dt.bfloat16, tag="fin")
                    nc.vector.tensor_tensor(_ap(u1, 0, 6, 1, 768), _ap(skel, 0, 6, 1, 768),
                                            _ap(other_x, 1, 6, 1, 768), A.mult)
                    z1 = dbl.tile([P_DIM, 6 * WS], dt.bfloat16, tag="fin")
                    nc.vector.tensor_tensor(_ap(z1, 0, 6, 1, 768), _ap(u1, 0, 6, 1, 768),
                                            _ap(r, 0, 6, 1, 768), A.mult)
                    nc.scalar.activation(_ap(scr, 0, 6, 1, 768), _ap(z1, 0, 6, 1, 768),
                                         F.Copy, accum_out=sums[:, s_base:s_base + 1])
                    z2 = dbl.tile([P_DIM, 6 * WS], dt.bfloat16, tag="fin")
                    nc.vector.tensor_tensor(_ap(z2, 0, 6, 1, 768), _ap(skel, 0, 6, 1, 768),
                                            _ap(r, 0, 6, 1, 768), A.mult)
                    nc.scalar.activation(_ap(scr, 0, 6, 1, 768), _ap(z2, 0, 6, 1, 768),
                                         F.Copy, accum_out=sums[:, s_base + 1:s_base + 2])
                    nc.scalar.activation(_ap(scr, 0, 6, 1, 768), _ap(u1, 0, 6, 1, 768),
                                         F.Copy, accum_out=sums[:, c1_col:c1_col + 1])

            out_t = pers.tile([P_DIM, n_img * NSUM], dt.float32)
            nc.vector.tensor_copy(out_t[:], sums[:])
            nc.sync.dma_start(sums_d[:], out_t[:])

    nc.compile()
    return nc


def host_layout(x):
    """(N,1,768,768) f32 -> (N,128,8,770) bf16 padded tiles."""
    n = x.shape[0]
    xb = np.ascontiguousarray(x.reshape(n, H, W)).astype(BF16)
    t = np.empty((n, P_DIM, NSLOT, WS), dtype=BF16)
    t[:, :, 1:7, 1:769] = xb.reshape(n, P_DIM, FOLD, W)
    t[:, 1:, 0, 1:769] = xb[:, 5::6][:, :127]      # halo row 6p-1
    t[:, 0, 0, 1:769] = xb[:, 0]                   # replicate row 0
    t[:, :127, 7, 1:769] = xb[:, 6::6]             # halo row 6p+6
    t[:, 127, 7, 1:769] = xb[:, 767]               # replicate row 767
    t[:, :, :, 0] = t[:, :, :, 1]                  # pad cols
    t[:, :, :, 769] = t[:, :, :, 768]
    return t


_NC_CACHE = {}


def _get_nc(n_img=4):
    if n_img not in _NC_CACHE:
        _NC_CACHE[n_img] = build(n_img=n_img)
    return _NC_CACHE[n_img]


def finish_host(all_sums):
    """all_sums: list of (128, n_img*NSUM) f32 per core -> scalar loss."""
    per_img = []
    for s in all_sums:
        v = s.astype(np.float64).sum(axis=0)
        for i in range(v.shape[0] // NSUM):
            per_img.append(v[i * NSUM:(i + 1) * NSUM])
    per_img = np.stack(per_img)
    s1, s2, s3, s4 = per_img[:, 0], per_img[:, 1], per_img[:, 2], per_img[:, 3]
    c0P, c1P, c0G, c1G = per_img[:, 4], per_img[:, 5], per_img[:, 6], per_img[:, 7]
    s1 = s1 + EPS * c1P
    s2 = s2 + EPS * c0P
    s3 = s3 + EPS * c1G
    s4 = s4 + EPS * c0G
    prec = s1 / (s2 + EPS)
    rec = s3 / (s4 + EPS)
    cb = 2.0 * prec * rec / (prec + rec + EPS)
    return np.float32(np.mean(1.0 - cb))


def kernel(pred, target):
    pred = np.asarray(pred)
    target = np.asarray(target)
    n_total = pred.shape[0]
    n_cores = 8
    n_img = n_total // n_cores
    nc = _get_nc(n_img)
    tp = host_layout(pred)
    tg = host_layout(target)
    in_maps = [{"pred": tp[c * n_img:(c + 1) * n_img],
                "targ": tg[c * n_img:(c + 1) * n_img]} for c in range(n_cores)]
    res = run_bass_kernel_spmd(nc, in_maps, list(range(n_cores)))
    return finish_host([r["sums"] for r in res.results])


# revision 13
# speedup vs baseline: 22.4890x; 1.1216x over previous
"""CBDice loss kernel for 8 TRN2 NeuronCores (Bass/Tile, bf16 pipeline).

Strategy
--------
Data parallel: 32 images sharded 4-per-core across 8 cores.

Per image and per tensor X in {pred, target} (all on-chip values bf16):
    e_0 = X;  e_t = erode3x3(e_{t-1})  t = 1..16      (shared erosion chain)
    d_t = dilate3x3(e_t)               t = 1..11
    soft_skeleton(X) = 1 - prod_{t=0..10} (1 - (e_t - d_{t+1}))
        (the reference's relu and clips are mathematically no-ops:
         opening is anti-extensive, and the recurrence stays in [0,1])
    radius(X) = (sum_{t=1..16} e_t / 16) * X          (clip is a no-op)
and per-image sums (f32, via ACT accum_out):
    s1 = sum(skelP*G*rG), s2 = sum(skelP*rG),
    s3 = sum(skelG*P*rP), s4 = sum(skelG*rP),
    c-sums of skel and skel*mask for exact EPS correction.
Final scalar math on host (f64).

Layout: one image-tensor = SBUF tile [128 partitions, 8 slots, 770 cols]
    slot s <-> image row 6p + s - 1 (slots 1..6 = data, 0/7 = halo rows)
    col  c <-> image col c - 1      (cols 1..768 = data, 0/769 = pads)
Halo rows / pad cols hold replicate padding (equivalent to the
reference's +-inf pooling pads for one 3x3 window). A 3x3 pool is 4 DVE
tensor_tensor ops (separable 3-taps, pairwise, bf16 2x mode). Halo rows
are refreshed after each erosion with 4 SBUF->SBUF DMAs (compute engines
cannot cross partitions); pad cols with 2 tiny ACT copies. ACT handles
all 1-input affine/copy/accum work. Host pre-builds the padded bf16
tiles so the device does zero layout work.
"""
import numpy as np
import ml_dtypes

import concourse.bass as bass
from concourse import bacc
import concourse.tile as tile
import concourse.mybir as mybir
from concourse.bass_utils import run_bass_kernel_spmd

A = mybir.AluOpType
F = mybir.ActivationFunctionType
dt = mybir.dt
BF16 = ml_dtypes.bfloat16

EPS = 1e-6
SKEL_ITERS = 10
RADIUS_ITERS = 16
H = W = 768
P_DIM = 128
FOLD = 6
NSLOT = 8
WS = 770
TILE_F = NSLOT * WS
NSUM = 10  # per-image sum columns (8 used)


def _ap(t, s0, nslots, c0=0, w=WS):
    """[(nslots),(w)] view of a [128, k*WS]-flat tile starting at slot s0, col c0."""
    if c0 == 0 and w == WS:
        return t[:, s0 * WS:(s0 + nslots) * WS]
    v = t[:, s0 * WS + c0: s0 * WS + c0 + 1]
    return bass.AP(tensor=v.tensor, offset=v.offset,
                   ap=[list(t[:, :].ap[0]), [WS, nslots], [1, w]])


def _ap_pair(t, sa, sb, c0, w):
    """[(2 slots sa,sb),(w)] strided view (sb>sa)."""
    v = t[:, sa * WS + c0: sa * WS + c0 + 1]
    return bass.AP(tensor=v.tensor, offset=v.offset,
                   ap=[list(t[:, :].ap[0]), [(sb - sa) * WS, 2], [1, w]])


def build(n_img=4, skel_iters=SKEL_ITERS, radius_iters=RADIUS_ITERS, repeat=1,
          skip_refresh=False, skip_skel=False, skip_acc=False, halo_hwdge=True):
    nc = bacc.Bacc("TRN2", target_bir_lowering=False, debug=False, num_devices=8)
    pred_d = nc.dram_tensor("pred", [n_img, P_DIM, NSLOT, WS], dt.bfloat16,
                            kind="ExternalInput").ap()
    targ_d = nc.dram_tensor("targ", [n_img, P_DIM, NSLOT, WS], dt.bfloat16,
                            kind="ExternalInput").ap()
    sums_d = nc.dram_tensor("sums", [P_DIM, n_img * NSUM], dt.float32,
                            kind="ExternalOutput").ap()
    n_dil = skel_iters + 1

    with tile.TileContext(nc) as tc:
        with tc.tile_pool(name="pers", bufs=1) as pers, \
             tc.tile_pool(name="dbl", bufs=2) as dbl, \
             tc.tile_pool(name="ech4", bufs=4) as ech4:
            sums = pers.tile([P_DIM, n_img * NSUM], dt.float32)
            nc.vector.memset(sums[:], 0.0)

            import contextlib
            rep_ctx = tc.For_i(0, repeat, 1) if repeat > 1 else contextlib.nullcontext()

            def pool3(e_src, out_data_ap, op, out_boundary_ap=None):
                """3x3 pool of 8-slot tile -> 6-slot data region; scrA holds
                A (7 slots), then tmp (in-place), then B (in-place fwd-shift)."""
                scrA = dbl.tile([P_DIM, 7 * WS], dt.bfloat16, tag="scrA")
                nc.vector.tensor_tensor(_ap(scrA, 0, 7), _ap(e_src, 0, 7),
                                        _ap(e_src, 1, 7), op)
                nc.vector.tensor_tensor(_ap(scrA, 0, 6), _ap(scrA, 0, 6),
                                        _ap(e_src, 2, 6), op)
                nc.vector.tensor_tensor(_ap(scrA, 0, 6, 0, 769),
                                        _ap(scrA, 0, 6, 0, 769),
                                        _ap(scrA, 0, 6, 1, 769), op)
                if out_boundary_ap is not None:
                    # boundary out slots first so halo DMAs launch early
                    nc.vector.tensor_tensor(out_boundary_ap,
                                            _ap_pair(scrA, 0, 5, 0, 768),
                                            _ap_pair(scrA, 0, 5, 1, 768), op)
                    nc.vector.tensor_tensor(out_data_ap,
                                            _ap(scrA, 1, 4, 0, 768),
                                            _ap(scrA, 1, 4, 1, 768), op)
                else:
                    nc.vector.tensor_tensor(out_data_ap,
                                            _ap(scrA, 0, 6, 0, 768),
                                            _ap(scrA, 0, 6, 1, 768), op)

            dma_eng = nc.sync if halo_hwdge else nc.gpsimd

            def refresh(e):
                """Restore pad cols (ACT) and halo rows (DMA) of chain tile e.

                Boundary slots {1,6} get pads first; the global-edge halo rows
                come from partition-aligned ACT copies (later overwritten on
                interior partitions by the big DMAs); the two big halo DMAs go
                out on different HWDGE queues (SP + ACT)."""
                if skip_refresh:
                    return
                nc.scalar.copy(_ap_pair(e, 1, 6, 0, 1), _ap_pair(e, 1, 6, 1, 1))
                nc.scalar.copy(_ap_pair(e, 1, 6, 769, 1), _ap_pair(e, 1, 6, 768, 1))
                nc.scalar.copy(_ap(e, 0, 1)[0:32, :], _ap(e, 1, 1)[0:32, :])
                nc.scalar.copy(_ap(e, 7, 1)[96:128, :], _ap(e, 6, 1)[96:128, :])
                nc.gpsimd.dma_start(_ap(e, 0, 1)[1:128, :], _ap(e, 6, 1)[0:127, :])
                nc.gpsimd.dma_start(_ap(e, 7, 1)[0:127, :], _ap(e, 1, 1)[1:128, :])
                # interior pad cols only feed the next H pass
                nc.scalar.copy(_ap(e, 2, 4, 0, 1), _ap(e, 2, 4, 1, 1))
                nc.scalar.copy(_ap(e, 2, 4, 769, 1), _ap(e, 2, 4, 768, 1))

            with rep_ctx:
              for n in range(n_img):
                xP = pers.tile([P_DIM, TILE_F], dt.bfloat16, tag="xP")
                nc.sync.dma_start(xP[:], pred_d[n].rearrange("p s w -> p (s w)"))
                xG = pers.tile([P_DIM, TILE_F], dt.bfloat16, tag="xG")
                nc.sync.dma_start(xG[:], targ_d[n].rearrange("p s w -> p (s w)"))

                st = {}
                for name, x0 in (("P", xP), ("G", xG)):
                    q_t = pers.tile([P_DIM, 6 * WS], dt.bfloat16, tag=f"q{name}")
                    acc_t = pers.tile([P_DIM, 6 * WS], dt.bfloat16, tag=f"acc{name}")
                    st[name] = {"x0": x0, "q": q_t, "acc": acc_t, "e_prev": x0}

                for t in range(1, radius_iters + 1):
                    # phase 1: both erodes (+halo refresh) first, so each
                    # side's refresh DMAs overlap the other side's pool work
                    for name in ("P", "G"):
                        S = st[name]
                        e_new = ech4.tile([P_DIM, TILE_F], dt.bfloat16, tag="ech")
                        pool3(S["e_prev"], _ap(e_new, 2, 4, 1, 768), A.min,
                              out_boundary_ap=_ap_pair(e_new, 1, 6, 1, 768))
                        refresh(e_new)
                        S["e_cur"] = e_new
                    # phase 2: dilates + skeleton + radius, then advance
                    for name in ("P", "G"):
                        S = st[name]
                        e_prev, e_new = S["e_prev"], S["e_cur"]
                        if t <= n_dil and not skip_skel:
                            d = dbl.tile([P_DIM, 6 * WS], dt.bfloat16, tag="d")
                            pool3(e_new, _ap(d, 1, 4, 1, 768), A.max,
                                  out_boundary_ap=_ap_pair(d, 0, 5, 1, 768))
                            t1 = dbl.tile([P_DIM, 6 * WS], dt.bfloat16, tag="pw")
                            nc.vector.tensor_tensor(_ap(t1, 0, 6, 1, 768),
                                                    _ap(d, 0, 6, 1, 768),
                                                    _ap(e_prev, 1, 6, 1, 768),
                                                    A.subtract)
                            q = S["q"]
                            if t == 1:
                                nc.scalar.activation(_ap(q, 0, 6, 1, 768),
                                                     _ap(t1, 0, 6, 1, 768),
                                                     F.Copy, bias=1.0, scale=1.0)
                            else:
                                u = dbl.tile([P_DIM, 6 * WS], dt.bfloat16, tag="pw")
                                nc.scalar.activation(_ap(u, 0, 6, 1, 768),
                                                     _ap(t1, 0, 6, 1, 768),
                                                     F.Copy, bias=1.0, scale=1.0)
                                nc.vector.tensor_tensor(_ap(q, 0, 6, 1, 768),
                                                        _ap(q, 0, 6, 1, 768),
                                                        _ap(u, 0, 6, 1, 768), A.mult)
                        acc = S["acc"]
                        if skip_acc:
                            pass
                        elif t == 1:
                            nc.scalar.copy(_ap(acc, 0, 6, 1, 768),
                                           _ap(e_new, 1, 6, 1, 768))
                        else:
                            nc.vector.tensor_tensor(_ap(acc, 0, 6, 1, 768),
                                                    _ap(acc, 0, 6, 1, 768),
                                                    _ap(e_new, 1, 6, 1, 768), A.add)
                        S["e_prev"] = e_new

                # ---- finals for this image ----
                base = n * NSUM
                scr = dbl.tile([P_DIM, 6 * WS], dt.bfloat16, tag="d")
                res = {}
                for name in ("P", "G"):
                    S = st[name]
                    x0, q, acc = S["x0"], S["q"], S["acc"]
                    skel = dbl.tile([P_DIM, 6 * WS], dt.bfloat16, tag="pw")
                    c0_col = base + (4 if name == "P" else 6)
                    nc.scalar.activation(_ap(skel, 0, 6, 1, 768), _ap(q, 0, 6, 1, 768),
                                         F.Copy, bias=1.0, scale=-1.0,
                                         accum_out=sums[:, c0_col:c0_col + 1])
                    rsc = dbl.tile([P_DIM, 6 * WS], dt.bfloat16, tag="fin")
                    nc.scalar.activation(_ap(rsc, 0, 6, 1, 768), _ap(acc, 0, 6, 1, 768),
                                         F.Copy, bias=0.0, scale=1.0 / radius_iters)
                    r = pers.tile([P_DIM, 6 * WS], dt.bfloat16, tag=f"acc{name}")
                    nc.vector.tensor_tensor(_ap(r, 0, 6, 1, 768), _ap(rsc, 0, 6, 1, 768),
                                            _ap(x0, 1, 6, 1, 768), A.mult)
                    res[name] = (x0, skel, r)

                xP0, skelP, rP = res["P"]
                xG0, skelG, rG = res["G"]
                for k, (skel, other_x, r, c1_col) in enumerate(
                        [(skelP, xG0, rG, base + 5), (skelG, xP0, rP, base + 7)]):
                    s_base = base + 2 * k
                    u1 = dbl.tile([P_DIM, 6 * WS], dt.bfloat16, tag="fin")
                    nc.vector.tensor_tensor(_ap(u1, 0, 6, 1, 768), _ap(skel, 0, 6, 1, 768),
                                            _ap(other_x, 1, 6, 1, 768), A.mult)
                    z1 = dbl.tile([P_DIM, 6 * WS], dt.bfloat16, tag="fin")
                    nc.vector.tensor_tensor(_ap(z1, 0, 6, 1, 768), _ap(u1, 0, 6, 1, 768),
                                            _ap(r, 0, 6, 1, 768), A.mult)
                    nc.scalar.activation(_ap(scr, 0, 6, 1, 768), _ap(z1, 0, 6, 1, 768),
                                         F.Copy, accum_out=sums[:, s_base:s_base + 1])
                    z2 = dbl.tile([P_DIM, 6 * WS], dt.bfloat16, tag="fin")
                    nc.vector.tensor_tensor(_ap(z2, 0, 6, 1, 768), _ap(skel, 0, 6, 1, 768),
                                            _ap(r, 0, 6, 1, 768), A.mult)
                    nc.scalar.activation(_ap(scr, 0, 6, 1, 768), _ap(z2, 0, 6, 1, 768),
                                         F.Copy, accum_out=sums[:, s_base + 1:s_base + 2])
                    nc.scalar.activation(_ap(scr, 0, 6, 1, 768), _ap(u1, 0, 6, 1, 768),
                                         F.Copy, accum_out=sums[:, c1_col:c1_col + 1])

            out_t = pers.tile([P_DIM, n_img * NSUM], dt.float32)
            nc.vector.tensor_copy(out_t[:], sums[:])
            nc.sync.dma_start(sums_d[:], out_t[:])

    nc.compile()
    return nc


def host_layout(x):
    """(N,1,768,768) f32 -> (N,128,8,770) bf16 padded tiles."""
    n = x.shape[0]
    xb = np.ascontiguousarray(x.reshape(n, H, W)).astype(BF16)
    t = np.empty((n, P_DIM, NSLOT, WS), dtype=BF16)
    t[:, :, 1:7, 1:769] = xb.reshape(n, P_DIM, FOLD, W)
    t[:, 1:, 0, 1:769] = xb[:, 5::6][:, :127]      # halo row 6p-1
    t[:, 0, 0, 1:769] = xb[:, 0]                   # replicate row 0
    t[:, :127, 7, 1:769] = xb[:, 6::6]             # halo row 6p+6
    t[:, 127, 7, 1:769] = xb[:, 767]               # replicate row 767
    t[:, :, :, 0] = t[:, :, :, 1]                  # pad cols
    t[:, :, :, 769] = t[:, :, :, 768]
    return t


_NC_CACHE = {}


def _get_nc(n_img=4):
    if n_img not in _NC_CACHE:
        _NC_CACHE[n_img] = build(n_img=n_img)
    return _NC_CACHE[n_img]


def finish_host(all_sums):
    """all_sums: list of (128, n_img*NSUM) f32 per core -> scalar loss."""
    per_img = []
    for s in all_sums:
        v = s.astype(np.float64).sum(axis=0)
        for i in range(v.shape[0] // NSUM):
            per_img.append(v[i * NSUM:(i + 1) * NSUM])
    per_img = np.stack(per_img)
    s1, s2, s3, s4 = per_img[:, 0], per_img[:, 1], per_img[:, 2], per_img[:, 3]
    c0P, c1P, c0G, c1G = per_img[:, 4], per_img[:, 5], per_img[:, 6], per_img[:, 7]
    s1 = s1 + EPS * c1P
    s2 = s2 + EPS * c0P
    s3 = s3 + EPS * c1G
    s4 = s4 + EPS * c0G
    prec = s1 / (s2 + EPS)
    rec = s3 / (s4 + EPS)
    cb = 2.0 * prec * rec / (prec + rec + EPS)
    return np.float32(np.mean(1.0 - cb))


def kernel(pred, target):
    pred = np.asarray(pred)
    target = np.asarray(target)
    n_total = pred.shape[0]
    n_cores = 8
    n_img = n_total // n_cores
    nc = _get_nc(n_img)
    tp = host_layout(pred)
    tg = host_layout(target)
    in_maps = [{"pred": tp[c * n_img:(c + 1) * n_img],
                "targ": tg[c * n_img:(c + 1) * n_img]} for c in range(n_cores)]
    res = run_bass_kernel_spmd(nc, in_maps, list(range(n_cores)))
    return finish_host([r["sums"] for r in res.results])
